# revision 26
# baseline (speedup 1.0000x reference)
"""Trainium2 Bass kernel for the attention-scoring module:

    out[b, s] = softmax_s( (enc[b] @ W.T + bias) @ h[b] )

Math: the bias term contributes a constant per (b, :) row, which cancels in
the softmax, and the two contractions reassociate:

    energies[b, s] = enc[b, s, :] . v[b]   with   v[b] = h[b] @ W

Sharding: data-parallel over batch — one batch per NeuronCore (B == 8 cores).
Per core: compute v with DVE multiply-accumulates + a Pool cross-partition
all-reduce (no PE matmuls, which run 4 cycles/row in fp32), stream enc[b]
(16 MB) through SBUF in ~1 MB DMA chunks (tapered at the end so the DVE
dot-product backlog drains before the last byte lands), do the dot-products
as fused multiply+row-sum DVE instructions, and run the softmax in column
blocks whose exp / partial sums overlap the stream (the tail exp covers a
single column).  The output leaves through a SWDGE writeback whose
descriptors are generated at kernel start and fired with a trigger at the
end, so the tail pays a ~1 ns trigger + transfer instead of a ~1.3 us HWDGE
DMA issue.
"""

from contextlib import ExitStack

import numpy as np

import concourse.tile as tile
from concourse import bacc, bass_isa, mybir
from concourse.bass_utils import run_bass_kernel_spmd
from concourse.masks import make_identity

B, S, H = 8, 8192, 512
N_CORES = 8
P = 128
N_COLS = S // P  # 64 energy columns, E[p, t] = energy(s = t*128 + p)
F32 = mybir.dt.float32
I32 = mybir.dt.int32
ALU = mybir.AluOpType
ACTF = mybir.ActivationFunctionType
AXX = mybir.AxisListType.X
RED = bass_isa.ReduceOp

# ~1 MB DMA chunks.  The taper keeps work(chunk k) <= transfer(chunk k+1)
# (594 ns of DVE per 128 rows vs 5.69 ns/row of DMA), so the last
# dot-product runs right when the last chunk's semaphore fires instead of
# behind a backlog.
CHUNK_ROWS = [512] * 12 + [384] + [256] * 4 + [128] * 5
CHUNK_BUFS = 16
# softmax exp runs per column block; the last block is a single column so
# only ~100 ns of exp remains after the final dot-product.
BLOCK_ENDS = [16, 32, 48, 63, 64]


def _build_kernel():
    nc = bacc.Bacc("TRN2", target_bir_lowering=False, debug=False)
    enc = nc.dram_tensor("enc", [S, H], F32, kind="ExternalInput")
    hvec = nc.dram_tensor("hvec", [1, H], F32, kind="ExternalInput")
    Wmat = nc.dram_tensor("W", [H, H], F32, kind="ExternalInput")
    # Padded to 2*S: the writeback ucode always walks 128 source partitions,
    # so partitions 64-127 (stale SBUF) land in the padding and are sliced
    # off on the host.
    out = nc.dram_tensor("out", [2 * S], F32, kind="ExternalOutput")

    dma_sem = nc.alloc_semaphore("out_dma")

    with ExitStack() as ctx:
        tc = ctx.enter_context(tile.TileContext(nc))
        consts = ctx.enter_context(tc.tile_pool(name="consts", bufs=1))
        small = ctx.enter_context(tc.tile_pool(name="small", bufs=1))
        chunks = ctx.enter_context(tc.tile_pool(name="chunks", bufs=CHUNK_BUFS))
        scratch = ctx.enter_context(tc.tile_pool(name="scratch", bufs=2))
        psum = ctx.enter_context(tc.tile_pool(name="psum", bufs=1, space="PSUM"))

        identity = consts.tile([P, P], F32)
        make_identity(nc, identity[:])
        one11 = consts.tile([1, 1], F32)
        nc.gpsimd.memset(one11[:], 1.0)
        ones_row = consts.tile([1, P], F32)
        nc.gpsimd.memset(ones_row[:], 1.0)
        ones_col = consts.tile([P, 1], F32)
        nc.gpsimd.memset(ones_col[:], 1.0)

        # probsT[t, :] will hold the seq range [t*128, (t+1)*128).
        probsT = small.tile([N_COLS, P], F32)
        ctx_idxs = consts.tile([P, 1], I32)
        nc.gpsimd.memset(ctx_idxs[:], 0)

        # ---- input DMAs: h (2 KB, k-on-partitions layout, on the ACT queue
        # so it doesn't delay W's issue), W (1 MB in two halves so the v
        # multiply-accumulates start earlier), then the enc chunks.
        hrow = small.tile([1, H], F32)
        nc.scalar.dma_start(hrow[:], hvec.ap())
        # h transposed into k-on-partitions layout via four free-size-1 PE
        # matvecs (a strided 4 B/descriptor DMA would cost ~220 ns of DMA
        # device time; this path is off the critical stream entirely).
        hT_ps = psum.tile([P, 4], F32, tag="hT")
        for c in range(4):
            nc.tensor.matmul(
                hT_ps[:, c : c + 1],
                hrow[:1, c * P : (c + 1) * P],
                one11[:],
                start=True,
                stop=True,
            )
        h_k = small.tile([P, 4], F32)  # h_k[p, c] = h[c*128 + p]
        nc.scalar.copy(h_k[:], hT_ps[:])
        W_sb = small.tile([P, 4, H], F32)  # W_sb[p, c, n] = W[c*128 + p, n]
        for half in range(2):
            nc.sync.dma_start(
                W_sb[:, 2 * half : 2 * half + 2, :],
                Wmat.ap()[2 * half * P : (2 * half + 2) * P, :].rearrange(
                    "(c p) n -> p c n", c=2, p=P
                ),
            )

        # ---- v = h @ W: per-partition multiply-accumulate over the four
        # k-blocks, then one cross-partition all-reduce (result lands on all
        # 128 partitions — no broadcast needed).
        acc = [small.tile([P, H], F32, tag=f"acc{i}", name=f"acc{i}") for i in range(2)]
        nc.vector.tensor_scalar_mul(acc[0][:], W_sb[:, 0, :], h_k[:, 0:1])
        for c in range(1, 4):
            nc.vector.scalar_tensor_tensor(
                out=acc[c % 2][:],
                in0=W_sb[:, c, :],
                scalar=h_k[:, c : c + 1],
                in1=acc[(c - 1) % 2][:],
                op0=ALU.mult,
                op1=ALU.add,
            )
        v_sb = small.tile([P, H], F32)
        nc.gpsimd.partition_all_reduce(v_sb[:], acc[1][:], channels=P, reduce_op=RED.add)

        # Trigger the ACT exp table load at t=0 instead of in the tail.
        dummy_act = small.tile([1, 1], F32)
        nc.scalar.activation(dummy_act[:], one11[:], ACTF.Exp, bias=0.0, scale=1.0)

        # ---- main loop: stream enc, fused multiply+reduce on DVE ----
        # Within each chunk: s = s0 + c*128 + p  ->  partition p, subtile c.
        E = small.tile([P, N_COLS], F32)
        P_exp = small.tile([P, N_COLS], F32)
        nblk = len(BLOCK_ENDS)
        rs = [
            small.tile([P, 1], F32, tag=f"rs{b}", name=f"rs{b}") for b in range(nblk)
        ]
        negM_sb = small.tile([P, 1], F32)
        S_ps = psum.tile([1, 1], F32, tag="sum")

        def emit_shift_chain():
            # Softmax shift from the first block's columns, computed
            # mid-stream.  Any shift within ~88 of the true max keeps exp()
            # finite, and the shift cancels exactly in the normalization.
            m_col = small.tile([P, 1], F32)
            nc.vector.tensor_reduce(m_col[:], E[:, : BLOCK_ENDS[0]], axis=AXX, op=ALU.max)
            m_all = small.tile([P, 1], F32)
            nc.gpsimd.partition_all_reduce(
                m_all[:], m_col[:], channels=P, reduce_op=RED.max
            )
            nc.vector.tensor_scalar_mul(negM_sb[:], m_all[:], -1.0)

        def emit_block(b):
            # exp + per-partition row-sum of block b's columns; partial-sum
            # the row-sums as soon as both operands exist.  The last (single
            # column) block skips ACT's 187 ns accumulator read; the idle DVE
            # row-sums it instead.
            lo = BLOCK_ENDS[b - 1] if b else 0
            hi = BLOCK_ENDS[b]
            last = b == nblk - 1
            nc.scalar.activation(
                P_exp[:, lo:hi],
                E[:, lo:hi],
                ACTF.Exp,
                bias=negM_sb[:],
                scale=1.0,
                accum_out=None if last else rs[b][:],
            )
            if last:
                # single-column row-sum on the (idle) DVE, skipping ACT's
                # 187 ns accumulator read
                nc.vector.tensor_reduce(rs[b][:], P_exp[:, lo:hi], axis=AXX, op=ALU.add)
            # fold the block's row-sums into the PSUM-accumulated total; only
            # this tiny free-size-1 matmul remains on the tail for the last
            # block.
            nc.tensor.matmul(
                S_ps[:], rs[b][:], ones_col[:], start=(b == 0), stop=last
            )

        s0 = 0
        next_block = 0
        E_half = small.tile([P, 1], F32)
        for k, rows in enumerate(CHUNK_ROWS):
            sub = rows // P
            is_last = k == len(CHUNK_ROWS) - 1
            ch = chunks.tile([P, sub, H], F32, tag="chunk")
            src = enc.ap()[s0 : s0 + rows, :].rearrange("(c p) h -> p c h", c=sub, p=P)
            if is_last:
                # split the final chunk's DMA and dot-product in half along H
                # so only a 297 ns half-STT plus a fused add trails the last
                # byte instead of a full 594 ns STT
                for hh in range(2):
                    nc.sync.dma_start(
                        ch[:, :, hh * (H // 2) : (hh + 1) * (H // 2)],
                        src[:, :, hh * (H // 2) : (hh + 1) * (H // 2)],
                    )
            else:
                nc.sync.dma_start(ch[:], src)
            for j in range(sub):
                prod = scratch.tile([P, H], F32, tag="prod")
                t = s0 // P + j
                if is_last:
                    nc.vector.scalar_tensor_tensor(
                        out=prod[:, : H // 2],
                        in0=ch[:, j, : H // 2],
                        scalar=1.0,
                        in1=v_sb[:, : H // 2],
                        op0=ALU.bypass,
                        op1=ALU.mult,
                        accum_out=E_half[:],
                    )
                    nc.vector.scalar_tensor_tensor(
                        out=prod[:, H // 2 :],
                        in0=ch[:, j, H // 2 :],
                        scalar=1.0,
                        in1=v_sb[:, H // 2 :],
                        op0=ALU.bypass,
                        op1=ALU.mult,
                        accum_out=E[:, t : t + 1],
                    )
                    # E[t] currently holds only the second half-sum; fold in
                    # the first half.
                    nc.vector.tensor_add(E[:, t : t + 1], E[:, t : t + 1], E_half[:])
                else:
                    # fused multiply + free-dim sum in one DVE instruction
                    nc.vector.scalar_tensor_tensor(
                        out=prod[:],
                        in0=ch[:, j, :],
                        scalar=1.0,
                        in1=v_sb[:],
                        op0=ALU.bypass,
                        op1=ALU.mult,
                        accum_out=E[:, t : t + 1],
                    )
                if next_block < nblk - 1 and t + 1 == BLOCK_ENDS[next_block]:
                    if next_block == 0:
                        emit_shift_chain()
                    emit_block(next_block)
                    next_block += 1
            s0 += rows

        # ---- softmax tail: exp of the last column, one add, one
        # all-reduce, 1/Σ, transpose, one scale, trigger the writeback.
        emit_block(nblk - 1)
        Sinv = small.tile([1, 1], F32)
        nc.vector.reciprocal(Sinv[:], S_ps[:])
        SinvB_ps = psum.tile([N_COLS, 1], F32, tag="sinvb")
        nc.tensor.matmul(
            SinvB_ps[:], ones_row[:1, :N_COLS], Sinv[:], start=True, stop=True
        )

        # PE warm-up: a small transpose (dep on the already-exp'd columns)
        # lifts the PE out of its lowest p-state so the real transpose runs
        # at the mid clock.
        warm_ps = psum.tile([8, N_COLS], F32, tag="warm")
        nc.tensor.transpose(
            warm_ps[:], P_exp[:N_COLS, 48:56], identity[:N_COLS, :N_COLS]
        )
        probsT_ps = psum.tile([N_COLS, P], F32, tag="outp")
        nc.tensor.transpose(probsT_ps[:], P_exp[:], identity[:])
        nc.vector.tensor_scalar_mul(probsT[:], probsT_ps[:], SinvB_ps[:])

        # Output writeback: the prep is emitted after probsT's writer so the
        # RAW edge exists and is deferred to the trigger; the prep itself
        # keeps only a no-sync edge, letting the scheduler hoist its ~1 us of
        # SWDGE descriptor generation into the stream.  The trigger in the
        # tail then just fires the descriptors.
        # The writeback ucode writes, for source partition p and dho index dd,
        # ncn contiguous floats at dst offset p*(d_head/128)*dho_stride + dd*
        # dho_stride, reading src partition p bytes [dd*ncn*4 ...).  Declaring
        # d_head=256 (dho=2), ncn=64 and a 256 B dho stride makes partition
        # p (= E column t) cover out[t*128, (t+1)*128) exactly — one
        # writeback for the whole output.
        nc.gpsimd.kv_writeback(
            out.ap().rearrange("(b i d n) -> b i d n", b=1, i=2 * N_COLS, d=2, n=P // 2),
            probsT[:].rearrange("t (d o n) -> t d o n", d=2, o=1, n=P // 2),
            ctx_idxs[:],
            prepare_only=True,
            sem=dma_sem,
        )
        nc.gpsimd.trigger_dma(count=None)
        nc.gpsimd.wait_ge(dma_sem, 16)

    nc.compile()

    # Tile's pool-cleanup waits for the writeback prep target its DMASW lane
    # sem, but a prepare_only prep's completion fires the caller-supplied
    # descriptor sem (out_dma) instead, so the lane sem never moves.  Retarget
    # those waits at out_dma — both mean "writeback landed" (+16 per DMA).
    dma_sem_id = None
    for blk in nc.m.functions[0].blocks:
        for inst in blk.instructions:
            si = inst.sync_info
            if si is None:
                continue
            for upd in si.on_update:
                if upd.ant_name == "out_dma":
                    dma_sem_id = upd.id
    assert dma_sem_id is not None
    for blk in nc.m.functions[0].blocks:
        for inst in blk.instructions:
            si = inst.sync_info
            if si is None:
                continue
            is_prep = type(inst).__name__ == "InstKVWritebackAnt"
            for ww in si.on_wait:
                if (ww.ant_name or "").startswith("DMASW"):
                    ww.id = dma_sem_id
                    ww.ant_name = "out_dma"
                    ww.wait_value = 0 if is_prep else 16

    # Tile lowers the trigger's deferred probsT dependency as an
    # EventSemaphore wait placed BEFORE the prep on the in-order Pool queue,
    # pinning the ~1 us descriptor generation behind the final scale.  Move
    # that wait onto the trigger itself, REPLACING the trigger's Pool
    # engine-tick wait: the tick wait only guards desc-gen completion, and
    # desc-gen (whose own deps clear by ~15 us) is always long done by the
    # time the scale (the last DVE op, ~54 us) fires the moved wait.
    from concourse import mybir as _mb

    pool_insts = [
        i
        for blk in nc.m.functions[0].blocks
        for i in blk.instructions
        if i.engine == _mb.EngineType.Pool
    ]
    prep_idx = next(
        k for k, i in enumerate(pool_insts) if type(i).__name__ == "InstKVWritebackAnt"
    )
    trig = next(i for i in pool_insts if type(i).__name__ == "InstTriggerDma")
    moved = None
    for k in range(max(0, prep_idx - 3), prep_idx):
        inst = pool_insts[k]
        if type(inst).__name__ != "InstEventSemaphore" or inst.sync_info is None:
            continue
        for ww in inst.sync_info.on_wait:
            nm = ww.ant_name or ""
            if nm.startswith(("DVE_", "Activation_", "PE_")):
                moved = (ww.id, nm, ww.wait_mode, ww.wait_value)
                ww.wait_value = 0
    if moved is not None:
        for tw in trig.sync_info.on_wait:
            if (tw.ant_name or "").startswith("Pool_"):
                tw.id, tw.ant_name, tw.wait_mode, tw.wait_value = moved

    return nc


_NC_CACHE = {}


def kernel(hidden, encoder_outputs, W, b):
    """Full (unsharded) inputs in, full output out; 8-core SPMD inside."""
    if "nc" not in _NC_CACHE:
        _NC_CACHE["nc"] = _build_kernel()
    nc = _NC_CACHE["nc"]

    hidden = np.asarray(hidden)
    enc = np.ascontiguousarray(np.asarray(encoder_outputs, dtype=np.float32))
    Wm = np.ascontiguousarray(np.asarray(W, dtype=np.float32))
    in_maps = [
        {
            "enc": enc[c],
            "hvec": np.ascontiguousarray(hidden[0, c][None, :].astype(np.float32)),
            "W": Wm,
        }
        for c in range(N_CORES)
    ]
    res = run_bass_kernel_spmd(nc, in_maps, core_ids=list(range(N_CORES)))
    return np.stack(
        [res.results[c]["out"][:S] for c in range(N_CORES)], axis=0
    ).astype(np.float32)
